# revision 1
# baseline (speedup 1.0000x reference)
"""DFT spectrogram (nn_DftSpectrogram) Bass kernel for 8 Trainium2 NeuronCores.

Pure data parallel: 32 batch items -> 4 per core. Per item (T=96512 samples):
  - 601 frames of 512 taps (stride 160) are loaded as 5 overlapping blocks of
    128 frames in [frame, tap] layout (contiguous 2KB rows -> efficient DMA)
  - folded with the DFT basis symmetry about tap 256: u[j]=x[j]+x[512-j]
    (cos side), v[j]=x[j]-x[512-j] (sin side) halve the matmul contraction to
    256; the j=0 term is folded into the ACT Square bias; the j=256 basis row
    is halved on the host to absorb the self-pairing
  - u/v are PE-transposed to [tap, frame] and matmul'd (fp32, exact) against
    the folded cos/sin bases (only k<256 is needed)
  - log-magnitude + per-frame mean/std normalization over the 256 freqs is
    done in [frame, k] layout: bn_stats/bn_aggr give mean+var in one DVE pass,
    the reduction axis (k) is the free dim
  - the normalized [frame, k] tiles are DMA'd straight out; the host fixes the
    layout to [k, frame] while gathering the 8 core shards

Engine balance per block: PE does 4 transposes + 4 fp32 matmuls; u-fold on
DVE, v-fold and r^2+i^2 on GPSIMD (otherwise idle), both squares (fused
PSUM->SBUF move) and ln on ACT, bn_stats on DVE, the PSUM->SBUF framesT copy
alternates ACT/DVE by block parity.

1/(sqrt(var)+eps') is computed entirely on DVE (int bit-trick seed + two
Heron steps) so ACT only ever needs the natural_log table set - one
ACT_TABLE_LOAD for the whole kernel instead of a ~2.7us switch per use of
Sqrt/Exp. eps compensation keeps the algebra exact:
(fft-mean)/(std+1e-7) == (g-mean_g)/(std_g+2*ln(10)*1e-7) for
fft = g * 0.5/ln(10).
"""
from contextlib import ExitStack

import numpy as np

import concourse.bass as bass
import concourse.tile as tile
from concourse import bacc, mybir
from concourse.bass_utils import run_bass_kernel_spmd

N_CORES = 8
B_FULL = 32
C_FULL = 1
T = 96512
NFFT = 512
KOUT = 256          # only lower half of the spectrum is kept
SHIFT = 160
F = (T - NFFT) // SHIFT + 1  # 601
BPC = B_FULL // N_CORES      # 4 items per core
EPS = 1e-7
CEPS = float(2.0 * np.log(10.0) * 1e-7)
F0S = (0, 128, 256, 384, 473)  # frame-block starts; last block overlaps by 39
FP32 = mybir.dt.float32
MM_DT = mybir.dt.float32  # exact fp32 matmul (f32r loses too much at nulls)


def _build(ctx: ExitStack, tc: "tile.TileContext", xh, wrh, wih, idh, outh,
           mm_dt, reps: int):
    nc = tc.nc
    AP = bass.AP
    AF = mybir.ActivationFunctionType

    consts = ctx.enter_context(tc.tile_pool(name="consts", bufs=1))
    fpool = ctx.enter_context(tc.tile_pool(name="frames", bufs=1))
    ftpool = ctx.enter_context(tc.tile_pool(name="framesT", bufs=4))
    mpool = ctx.enter_context(tc.tile_pool(name="mag", bufs=8))
    glpool = ctx.enter_context(tc.tile_pool(name="gl", bufs=4))
    spool = ctx.enter_context(tc.tile_pool(name="stats", bufs=6))
    gnpool = ctx.enter_context(tc.tile_pool(name="gnorm", bufs=4))
    ptrp = ctx.enter_context(tc.tile_pool(name="ptr", bufs=3, space="PSUM"))
    prip = ctx.enter_context(tc.tile_pool(name="pri", bufs=5, space="PSUM"))

    c_sb = consts.tile([128, 2 * KOUT], FP32, tag="c_sb")
    s_sb = consts.tile([128, 2 * KOUT], FP32, tag="s_sb")
    ident = consts.tile([128, 128], FP32, tag="ident")
    epsb = consts.tile([128, 1], FP32, tag="epsb")
    nc.vector.memset(epsb[:], EPS)

    def stage_front(b, ftile, fb, gl, mv3):
        """u/v fold, transposes, matmuls, squares, ln, bn stats for one block."""
        src_f = ftile[:, fb * NFFT:(fb + 1) * NFFT]
        # fold: u[j] = x[j] + x[512-j], v[j] = x[j] - x[512-j], j=1..256
        fwd = src_f[:, 1:257]
        rev = src_f[:, 511:255:-1]
        u = mpool.tile([128, KOUT], FP32, tag="u", name="u")
        nc.vector.tensor_add(u[:], fwd, rev)
        v = mpool.tile([128, KOUT], FP32, tag="v", name="v")
        nc.gpsimd.tensor_sub(v[:], fwd, rev)

        # transpose u,v [128f, 256j] -> 4x [128j, 128f] in one PSUM bank
        ptr = ptrp.tile([128, NFFT], FP32, tag="ptr", name="ptr")
        for c, srcc in enumerate((u[:, 0:128], u[:, 128:256],
                                  v[:, 0:128], v[:, 128:256])):
            nc.tensor.matmul(ptr[:, c * 128:(c + 1) * 128],
                             srcc, ident[:], is_transpose=True,
                             start=(c == 0), stop=(c == 3))
        ft_sb = ftpool.tile([128, NFFT], FP32, tag="ft_sb", name="ft_sb")
        nc.scalar.copy(ft_sb[:, 0:KOUT], ptr[:, 0:KOUT])
        nc.vector.tensor_copy(ft_sb[:, KOUT:2 * KOUT], ptr[:, KOUT:2 * KOUT])

        # real[f,k] = sum_j u[f,j] C[j,k] (+ x[160f], via Square bias)
        # imag[f,k] = sum_j v[f,j] S[j,k]
        pri = prip.tile([128, 2 * KOUT], FP32, tag="pri", name="pri")
        nc.tensor.matmul(pri[:, 0:KOUT], ft_sb[:, 0:128],
                         c_sb[:, 0:KOUT], start=True, stop=False)
        nc.tensor.matmul(pri[:, 0:KOUT], ft_sb[:, 128:256],
                         c_sb[:, KOUT:2 * KOUT], start=False, stop=False)
        nc.tensor.matmul(pri[:, KOUT:2 * KOUT], ft_sb[:, 256:384],
                         s_sb[:, 0:KOUT], start=False, stop=False)
        nc.tensor.matmul(pri[:, KOUT:2 * KOUT], ft_sb[:, 384:512],
                         s_sb[:, KOUT:2 * KOUT], start=False, stop=True)

        sq = mpool.tile([128, 2 * KOUT], FP32, tag="sq", name="sq")
        nc.scalar.activation(sq[:, 0:KOUT], pri[:, 0:KOUT], AF.Square,
                             bias=src_f[:, 0:1])
        nc.scalar.activation(sq[:, KOUT:2 * KOUT], pri[:, KOUT:2 * KOUT],
                             AF.Square)
        msum = mpool.tile([128, KOUT], FP32, tag="msum", name="msum")
        nc.gpsimd.tensor_add(msum[:], sq[:, 0:KOUT], sq[:, KOUT:2 * KOUT])
        gls = gl[:, fb * KOUT:(fb + 1) * KOUT]
        nc.scalar.activation(gls, msum[:], AF.Ln, bias=epsb[:])
        bn6 = spool.tile([128, 6], FP32, tag="bn6", name="bn6")
        nc.vector.bn_stats(bn6[:], gls)
        nc.vector.bn_aggr(mv3[:, fb, :], bn6[:])

    def stage_back(b, gl, mv, fb_lo=0, fb_hi=4):
        """rden = 1/(sqrt(var)+ceps) on DVE only (int bit-trick sqrt seed +
        two Heron steps, 5e-7 rel; keeps ACT on one table set), then
        normalize and DMA out, for blocks fb_lo..fb_hi of one item. Emitted
        late so the serial tiny-op chain sits behind the next blocks'
        PE-feeding work in the DVE FIFO."""
        w = fb_hi - fb_lo + 1
        var = bass.AP(mv[:].tensor, mv[:].offset + 2 * fb_lo + 1,
                      [list(mv[:].ap[0]), [2, w]])
        sh = spool.tile([128, w], mybir.dt.int32, tag="sh", name="sh")
        nc.vector.tensor_scalar(sh[:], var.bitcast(mybir.dt.int32), 1, None,
                                op0=mybir.AluOpType.arith_shift_right)
        s0i = spool.tile([128, w], mybir.dt.int32, tag="s0i", name="s0i")
        nc.vector.tensor_scalar(s0i[:], sh[:], 0x1FBD1DF5, None,
                                op0=mybir.AluOpType.add)
        s_cur = s0i[:].bitcast(FP32)
        for it in range(2):
            hr = spool.tile([128, w], FP32, tag=f"hr{it}", name=f"hr{it}")
            nc.vector.reciprocal(hr[:], s_cur)
            ht = spool.tile([128, w], FP32, tag=f"ht{it}", name=f"ht{it}")
            nc.vector.tensor_mul(ht[:], var, hr[:])
            hs = spool.tile([128, w], FP32, tag=f"hs{it}", name=f"hs{it}")
            nc.vector.tensor_add(hs[:], s_cur, ht[:])
            hh = spool.tile([128, w], FP32, tag=f"hh{it}", name=f"hh{it}")
            nc.vector.tensor_scalar_mul(hh[:], hs[:], 0.5)
            s_cur = hh[:]
        uu = spool.tile([128, w], FP32, tag="uu", name="uu")
        nc.vector.tensor_scalar(uu[:], s_cur, 1.0, CEPS,
                                op0=mybir.AluOpType.mult,
                                op1=mybir.AluOpType.add)
        rden = spool.tile([128, w], FP32, tag="rden", name="rden")
        nc.vector.reciprocal(rden[:], uu[:])

        # normalize into one tile so blocks 0..3 leave in a single strided
        # DMA (out[b, fb*128+p, k] <- gn4[p, fb, k]) instead of four serial
        # HWDGE transfers; block 4 (only 89 new frames) goes separately
        gn4 = None
        for fb in range(fb_lo, fb_hi + 1):
            gls = gl[:, fb * KOUT:(fb + 1) * KOUT]
            if fb < 4:
                if gn4 is None:
                    gn4 = gnpool.tile([128, 4 * KOUT], FP32, tag="gn4",
                                      name="gn4")
                gdst = gn4[:, fb * KOUT:(fb + 1) * KOUT]
            else:
                gdst = gnpool.tile([128, KOUT], FP32, tag="gn", name="gn")
            nc.vector.tensor_scalar(gdst, gls,
                                    mv[:, 2 * fb:2 * fb + 1],
                                    rden[:, fb - fb_lo:fb - fb_lo + 1],
                                    op0=mybir.AluOpType.subtract,
                                    op1=mybir.AluOpType.mult)
            if fb == 4:
                # frames 473..511 were already written by block 3
                nc.sync.dma_start(outh.ap()[b, 512:601, :], gdst[39:128, :])
        if gn4 is not None:
            nblk = min(fb_hi, 3) - fb_lo + 1
            dst = bass.AP(outh, b * F * KOUT + fb_lo * 128 * KOUT,
                          [[KOUT, 128], [128 * KOUT, nblk], [1, KOUT]])
            nc.sync.dma_start(
                dst, gn4[:, fb_lo * KOUT:(fb_lo + nblk) * KOUT].rearrange(
                    "p (f k) -> p f k", k=KOUT))

    def body():
        # issue every input DMA first: otherwise item b+1's loads queue on
        # the SP HWDGE ring behind item b's dep-gated output DMAs. The very
        # first frame chunk goes before the constants - it gates the first
        # transpose, while the bases are only needed ~3us in.
        ftiles = [fpool.tile([128, 5 * NFFT], FP32, tag=f"ftile{b}",
                             name=f"ftile{b}") for b in range(BPC)]

        def fdma(b, fb):
            srcb = AP(xh, b * T + SHIFT * F0S[fb], [[SHIFT, 128], [1, NFFT]])
            nc.sync.dma_start(ftiles[b][:, fb * NFFT:(fb + 1) * NFFT], srcb)

        fdma(0, 0)
        if reps == 1:
            # constants after the first (PE-gating) frame chunk; for the
            # timing loop variant they are loaded once before the loop
            nc.sync.dma_start(ident[:], idh.ap())
            nc.sync.dma_start(c_sb[:].rearrange("p (c k) -> p c k", k=KOUT),
                              wrh.ap().rearrange("(c p) k -> p c k", p=128))
            nc.sync.dma_start(s_sb[:].rearrange("p (c k) -> p c k", k=KOUT),
                              wih.ap().rearrange("(c p) k -> p c k", p=128))
        fdma(0, 1)
        for b in range(BPC):
            for fb in range(5):
                if (b, fb) not in ((0, 0), (0, 1)):
                    fdma(b, fb)

        pending = None
        for b in range(BPC):
            last = b == BPC - 1
            gl = glpool.tile([128, 5 * KOUT], FP32, tag="gl", name="gl")
            mv = spool.tile([128, 10], FP32, tag="mv", name="mv")
            mv3 = mv[:].rearrange("p (f two) -> p f two", two=2)
            for fb in range(5):
                stage_front(b, ftiles[b], fb, gl, mv3)
                if fb == 3 and pending is not None:
                    stage_back(*pending)
                    pending = None
                if last and fb == 3:
                    # split the final item's normalization so only block 4's
                    # short chain sits in the kernel tail; blocks 0..3 drain
                    # while block 4 is still in its matmuls
                    stage_back(b, gl, mv, 0, 3)
            if not last:
                pending = (b, gl, mv)
            else:
                stage_back(b, gl, mv, 4, 4)

    if reps == 1:
        body()
    else:
        nc.sync.dma_start(ident[:], idh.ap())
        nc.sync.dma_start(c_sb[:].rearrange("p (c k) -> p c k", k=KOUT),
                          wrh.ap().rearrange("(c p) k -> p c k", p=128))
        nc.sync.dma_start(s_sb[:].rearrange("p (c k) -> p c k", k=KOUT),
                          wih.ap().rearrange("(c p) k -> p c k", p=128))
        with tc.For_i(0, reps, 1):
            body()


def build_nc(mm_dt=MM_DT, reps: int = 1):
    nc = bacc.Bacc("TRN2", target_bir_lowering=False, debug=False)
    xh = nc.dram_tensor("x", [BPC, T], FP32, kind="ExternalInput")
    wrh = nc.dram_tensor("wr", [KOUT, KOUT], FP32, kind="ExternalInput")
    wih = nc.dram_tensor("wi", [KOUT, KOUT], FP32, kind="ExternalInput")
    idh = nc.dram_tensor("ident", [128, 128], FP32, kind="ExternalInput")
    outh = nc.dram_tensor("out", [BPC, F, KOUT], FP32, kind="ExternalOutput")
    with tile.TileContext(nc) as tc, ExitStack() as ctx:
        _build(ctx, tc, xh, wrh, wih, idh, outh, mm_dt, reps)
    nc.compile()
    return nc


def make_in_maps(x, W_real, W_imag):
    xs = np.asarray(x, dtype=np.float32).reshape(B_FULL, T)
    Wr = np.asarray(W_real, np.float32)
    Wi = np.asarray(W_imag, np.float32)
    # folded bases, rows j=1..256; j=256 halved (cos) / zero (sin, exact)
    wr_dev = np.zeros((KOUT, KOUT), np.float32)
    wi_dev = np.zeros((KOUT, KOUT), np.float32)
    wr_dev[:255] = Wr[:KOUT, 1:256].T
    wr_dev[255] = 0.5 * Wr[:KOUT, 256]
    wi_dev[:255] = Wi[:KOUT, 1:256].T
    wi_dev[255] = 0.0
    ident = np.eye(128, dtype=np.float32)
    return [
        {"x": np.ascontiguousarray(xs[i * BPC:(i + 1) * BPC]),
         "wr": wr_dev, "wi": wi_dev, "ident": ident}
        for i in range(N_CORES)
    ]


_NC_CACHE = {}


def kernel(x, W_real, W_imag):
    key = (str(MM_DT), 1)
    if key not in _NC_CACHE:
        _NC_CACHE[key] = build_nc(MM_DT, 1)
    nc = _NC_CACHE[key]
    in_maps = make_in_maps(x, W_real, W_imag)
    res = run_bass_kernel_spmd(nc, in_maps, core_ids=list(range(N_CORES)))
    out = np.concatenate([r["out"] for r in res.results], axis=0)  # [32, F, K]
    out = np.ascontiguousarray(out.transpose(0, 2, 1))             # [32, K, F]
    return out.reshape(B_FULL, C_FULL, KOUT, F).astype(np.float32)



# revision 6
# speedup vs baseline: 1.0309x; 1.0309x over previous
"""DFT spectrogram (nn_DftSpectrogram) Bass kernel for 8 Trainium2 NeuronCores.

Pure data parallel: 32 batch items -> 4 per core. Per item (T=96512 samples):
  - 601 frames of 512 taps (stride 160) are loaded as 5 overlapping blocks of
    128 frames in [frame, tap] layout (contiguous 2KB rows -> efficient DMA)
  - folded with the DFT basis symmetry about tap 256: u[j]=x[j]+x[512-j]
    (cos side), v[j]=x[j]-x[512-j] (sin side) halve the matmul contraction to
    256; the j=0 term is folded into the ACT Square bias; the j=256 basis row
    is halved on the host to absorb the self-pairing
  - u/v are PE-transposed to [tap, frame] and matmul'd (fp32, exact) against
    the folded cos/sin bases (only k<256 is needed)
  - log-magnitude + per-frame mean/std normalization over the 256 freqs is
    done in [frame, k] layout: bn_stats/bn_aggr give mean+var in one DVE pass,
    the reduction axis (k) is the free dim
  - the normalized [frame, k] tiles are DMA'd straight out; the host fixes the
    layout to [k, frame] while gathering the 8 core shards

Engine balance per block: PE does 4 transposes + 4 fp32 matmuls; u-fold on
DVE, v-fold and r^2+i^2 on GPSIMD (otherwise idle), both squares (fused
PSUM->SBUF move) and ln on ACT, bn_stats on DVE, the PSUM->SBUF framesT copy
alternates ACT/DVE by block parity.

1/(sqrt(var)+eps') is computed entirely on DVE (int bit-trick seed + two
Heron steps) so ACT only ever needs the natural_log table set - one
ACT_TABLE_LOAD for the whole kernel instead of a ~2.7us switch per use of
Sqrt/Exp. eps compensation keeps the algebra exact:
(fft-mean)/(std+1e-7) == (g-mean_g)/(std_g+2*ln(10)*1e-7) for
fft = g * 0.5/ln(10).
"""
from contextlib import ExitStack

import numpy as np

import concourse.bass as bass
import concourse.tile as tile
from concourse import bacc, mybir
from concourse.bass_utils import run_bass_kernel_spmd

N_CORES = 8
B_FULL = 32
C_FULL = 1
T = 96512
NFFT = 512
KOUT = 256          # only lower half of the spectrum is kept
SHIFT = 160
F = (T - NFFT) // SHIFT + 1  # 601
BPC = B_FULL // N_CORES      # 4 items per core
EPS = 1e-7
CEPS = float(2.0 * np.log(10.0) * 1e-7)
F0S = (0, 128, 256, 384, 473)  # frame-block starts; last block overlaps by 39
FP32 = mybir.dt.float32
F32R = mybir.dt.float32r
# f32r: PE runs 1 cycle/row (vs 4 for exact fp32) when the moving dim is
# >= 256; data stays fp32 in SBUF, only the PE datapath rounds. The l2
# correctness gate is 2e-2, measured rel err with f32r is checked on HW.
MM_DT = F32R


def _build(ctx: ExitStack, tc: "tile.TileContext", xh, wrh, wih, idh, outh,
           mm_dt, reps: int):
    nc = tc.nc
    AP = bass.AP
    AF = mybir.ActivationFunctionType

    MDT = mm_dt  # dtype of all PE operands (f32r: 11-bit-mantissa fp32)

    consts = ctx.enter_context(tc.tile_pool(name="consts", bufs=1))
    fpool = ctx.enter_context(tc.tile_pool(name="frames", bufs=1))
    ftpool = ctx.enter_context(tc.tile_pool(name="framesT", bufs=4))
    mpool = ctx.enter_context(tc.tile_pool(name="mag", bufs=8))
    glpool = ctx.enter_context(tc.tile_pool(name="gl", bufs=4))
    spool = ctx.enter_context(tc.tile_pool(name="stats", bufs=6))
    gnpool = ctx.enter_context(tc.tile_pool(name="gnorm", bufs=4))
    ptrp = ctx.enter_context(tc.tile_pool(name="ptr", bufs=3, space="PSUM"))
    prip = ctx.enter_context(tc.tile_pool(name="pri", bufs=5, space="PSUM"))

    c_sb = consts.tile([128, 2 * KOUT], MDT, tag="c_sb")
    s_sb = consts.tile([128, 2 * KOUT], MDT, tag="s_sb")
    ident = consts.tile([128, 128], MDT, tag="ident")
    epsb = consts.tile([128, 1], FP32, tag="epsb")
    nc.vector.memset(epsb[:], EPS)

    def stage_front(b, ftile, fb, gl, mv3):
        """u/v fold, transposes, matmuls, squares, ln, bn stats for one block."""
        src_f = ftile[:, fb * NFFT:(fb + 1) * NFFT]
        # fold: u[j] = x[j] + x[512-j], v[j] = x[j] - x[512-j], j=1..256
        fwd = src_f[:, 1:257]
        rev = src_f[:, 511:255:-1]
        u = mpool.tile([128, KOUT], MDT, tag="u", name="u")
        nc.vector.tensor_add(u[:], fwd, rev)
        v = mpool.tile([128, KOUT], MDT, tag="v", name="v")
        nc.gpsimd.tensor_sub(v[:], fwd, rev)

        # transpose u,v [128f, 256j] -> 4x [128j, 128f] in one PSUM bank
        ptr = ptrp.tile([128, NFFT], MDT, tag="ptr", name="ptr")
        for c, srcc in enumerate((u[:, 0:128], u[:, 128:256],
                                  v[:, 0:128], v[:, 128:256])):
            nc.tensor.matmul(ptr[:, c * 128:(c + 1) * 128],
                             srcc, ident[:], is_transpose=True,
                             start=(c == 0), stop=(c == 3))
        ft_sb = ftpool.tile([128, NFFT], MDT, tag="ft_sb", name="ft_sb")
        nc.scalar.copy(ft_sb[:, 0:KOUT], ptr[:, 0:KOUT])
        nc.vector.tensor_copy(ft_sb[:, KOUT:2 * KOUT], ptr[:, KOUT:2 * KOUT])

        # real[f,k] = sum_j u[f,j] C[j,k] (+ x[160f], via Square bias)
        # imag[f,k] = sum_j v[f,j] S[j,k]
        pri = prip.tile([128, 2 * KOUT], FP32, tag="pri", name="pri")
        nc.tensor.matmul(pri[:, 0:KOUT], ft_sb[:, 0:128],
                         c_sb[:, 0:KOUT], start=True, stop=False)
        nc.tensor.matmul(pri[:, 0:KOUT], ft_sb[:, 128:256],
                         c_sb[:, KOUT:2 * KOUT], start=False, stop=False)
        nc.tensor.matmul(pri[:, KOUT:2 * KOUT], ft_sb[:, 256:384],
                         s_sb[:, 0:KOUT], start=False, stop=False)
        nc.tensor.matmul(pri[:, KOUT:2 * KOUT], ft_sb[:, 384:512],
                         s_sb[:, KOUT:2 * KOUT], start=False, stop=True)

        sq = mpool.tile([128, 2 * KOUT], FP32, tag="sq", name="sq")
        nc.scalar.activation(sq[:, 0:KOUT], pri[:, 0:KOUT], AF.Square,
                             bias=src_f[:, 0:1])
        nc.scalar.activation(sq[:, KOUT:2 * KOUT], pri[:, KOUT:2 * KOUT],
                             AF.Square)
        msum = mpool.tile([128, KOUT], FP32, tag="msum", name="msum")
        nc.gpsimd.tensor_add(msum[:], sq[:, 0:KOUT], sq[:, KOUT:2 * KOUT])
        gls = gl[:, fb * KOUT:(fb + 1) * KOUT]
        nc.scalar.activation(gls, msum[:], AF.Ln, bias=epsb[:])
        bn6 = spool.tile([128, 6], FP32, tag="bn6", name="bn6")
        nc.vector.bn_stats(bn6[:], gls)
        nc.vector.bn_aggr(mv3[:, fb, :], bn6[:])

    def stage_back(b, gl, mv, fb_lo=0, fb_hi=4):
        """rden = 1/(sqrt(var)+ceps) on DVE only (int bit-trick sqrt seed +
        two Heron steps, 5e-7 rel; keeps ACT on one table set), then
        normalize and DMA out, for blocks fb_lo..fb_hi of one item. Emitted
        late so the serial tiny-op chain sits behind the next blocks'
        PE-feeding work in the DVE FIFO."""
        w = fb_hi - fb_lo + 1
        var = bass.AP(mv[:].tensor, mv[:].offset + 2 * fb_lo + 1,
                      [list(mv[:].ap[0]), [2, w]])
        sh = spool.tile([128, w], mybir.dt.int32, tag="sh", name="sh")
        nc.vector.tensor_scalar(sh[:], var.bitcast(mybir.dt.int32), 1, None,
                                op0=mybir.AluOpType.arith_shift_right)
        s0i = spool.tile([128, w], mybir.dt.int32, tag="s0i", name="s0i")
        nc.vector.tensor_scalar(s0i[:], sh[:], 0x1FBD1DF5, None,
                                op0=mybir.AluOpType.add)
        s_cur = s0i[:].bitcast(FP32)
        for it in range(2):
            hr = spool.tile([128, w], FP32, tag=f"hr{it}", name=f"hr{it}")
            nc.vector.reciprocal(hr[:], s_cur)
            ht = spool.tile([128, w], FP32, tag=f"ht{it}", name=f"ht{it}")
            nc.vector.tensor_mul(ht[:], var, hr[:])
            hs = spool.tile([128, w], FP32, tag=f"hs{it}", name=f"hs{it}")
            nc.vector.tensor_add(hs[:], s_cur, ht[:])
            hh = spool.tile([128, w], FP32, tag=f"hh{it}", name=f"hh{it}")
            nc.vector.tensor_scalar_mul(hh[:], hs[:], 0.5)
            s_cur = hh[:]
        uu = spool.tile([128, w], FP32, tag="uu", name="uu")
        nc.vector.tensor_scalar(uu[:], s_cur, 1.0, CEPS,
                                op0=mybir.AluOpType.mult,
                                op1=mybir.AluOpType.add)
        rden = spool.tile([128, w], FP32, tag="rden", name="rden")
        nc.vector.reciprocal(rden[:], uu[:])

        # normalize into one tile so blocks 0..3 leave in a single strided
        # DMA (out[b, fb*128+p, k] <- gn4[p, fb, k]) instead of four serial
        # HWDGE transfers; block 4 (only 89 new frames) goes separately
        gn4 = None
        for fb in range(fb_lo, fb_hi + 1):
            gls = gl[:, fb * KOUT:(fb + 1) * KOUT]
            if fb < 4:
                if gn4 is None:
                    gn4 = gnpool.tile([128, 4 * KOUT], FP32, tag="gn4",
                                      name="gn4")
                gdst = gn4[:, fb * KOUT:(fb + 1) * KOUT]
            else:
                gdst = gnpool.tile([128, KOUT], FP32, tag="gn", name="gn")
            nc.vector.tensor_scalar(gdst, gls,
                                    mv[:, 2 * fb:2 * fb + 1],
                                    rden[:, fb - fb_lo:fb - fb_lo + 1],
                                    op0=mybir.AluOpType.subtract,
                                    op1=mybir.AluOpType.mult)
            if fb == 4:
                # frames 473..511 were already written by block 3
                nc.sync.dma_start(outh.ap()[b, 512:601, :], gdst[39:128, :])
        if gn4 is not None:
            nblk = min(fb_hi, 3) - fb_lo + 1
            dst = bass.AP(outh, b * F * KOUT + fb_lo * 128 * KOUT,
                          [[KOUT, 128], [128 * KOUT, nblk], [1, KOUT]])
            nc.sync.dma_start(
                dst, gn4[:, fb_lo * KOUT:(fb_lo + nblk) * KOUT].rearrange(
                    "p (f k) -> p f k", k=KOUT))

    def body():
        # issue every input DMA first: otherwise item b+1's loads queue on
        # the SP HWDGE ring behind item b's dep-gated output DMAs. The very
        # first frame chunk goes before the constants - it gates the first
        # transpose, while the bases are only needed ~3us in.
        ftiles = [fpool.tile([128, 5 * NFFT], FP32, tag=f"ftile{b}",
                             name=f"ftile{b}") for b in range(BPC)]

        def fdma(b, fb):
            srcb = AP(xh, b * T + SHIFT * F0S[fb], [[SHIFT, 128], [1, NFFT]])
            nc.sync.dma_start(ftiles[b][:, fb * NFFT:(fb + 1) * NFFT], srcb)

        fdma(0, 0)
        if reps == 1:
            # constants after the first (PE-gating) frame chunk; for the
            # timing loop variant they are loaded once before the loop
            nc.sync.dma_start(ident[:], idh.ap().bitcast(MDT))
            nc.sync.dma_start(c_sb[:].rearrange("p (c k) -> p c k", k=KOUT),
                              wrh.ap().rearrange("(c p) k -> p c k",
                                                 p=128).bitcast(MDT))
            nc.sync.dma_start(s_sb[:].rearrange("p (c k) -> p c k", k=KOUT),
                              wih.ap().rearrange("(c p) k -> p c k",
                                                 p=128).bitcast(MDT))
        fdma(0, 1)
        for b in range(BPC):
            for fb in range(5):
                if (b, fb) not in ((0, 0), (0, 1)):
                    fdma(b, fb)

        pending = None
        for b in range(BPC):
            last = b == BPC - 1
            gl = glpool.tile([128, 5 * KOUT], FP32, tag="gl", name="gl")
            mv = spool.tile([128, 10], FP32, tag="mv", name="mv")
            mv3 = mv[:].rearrange("p (f two) -> p f two", two=2)
            for fb in range(5):
                stage_front(b, ftiles[b], fb, gl, mv3)
                if fb == 3 and pending is not None:
                    stage_back(*pending)
                    pending = None
                if last and fb == 3:
                    # split the final item's normalization so only block 4's
                    # short chain sits in the kernel tail; blocks 0..3 drain
                    # while block 4 is still in its matmuls
                    stage_back(b, gl, mv, 0, 3)
            if not last:
                pending = (b, gl, mv)
            else:
                stage_back(b, gl, mv, 4, 4)

    if reps == 1:
        body()
    else:
        nc.sync.dma_start(ident[:], idh.ap().bitcast(MDT))
        nc.sync.dma_start(c_sb[:].rearrange("p (c k) -> p c k", k=KOUT),
                          wrh.ap().rearrange("(c p) k -> p c k",
                                             p=128).bitcast(MDT))
        nc.sync.dma_start(s_sb[:].rearrange("p (c k) -> p c k", k=KOUT),
                          wih.ap().rearrange("(c p) k -> p c k",
                                             p=128).bitcast(MDT))
        with tc.For_i(0, reps, 1):
            body()


def build_nc(mm_dt=MM_DT, reps: int = 1):
    nc = bacc.Bacc("TRN2", target_bir_lowering=False, debug=False)
    xh = nc.dram_tensor("x", [BPC, T], FP32, kind="ExternalInput")
    wrh = nc.dram_tensor("wr", [KOUT, KOUT], FP32, kind="ExternalInput")
    wih = nc.dram_tensor("wi", [KOUT, KOUT], FP32, kind="ExternalInput")
    idh = nc.dram_tensor("ident", [128, 128], FP32, kind="ExternalInput")
    outh = nc.dram_tensor("out", [BPC, F, KOUT], FP32, kind="ExternalOutput")
    with tile.TileContext(nc) as tc, ExitStack() as ctx:
        _build(ctx, tc, xh, wrh, wih, idh, outh, mm_dt, reps)
    nc.compile()
    return nc


def _round_f32r(a):
    """Round fp32 to f32r (1-8-11): round-to-nearest-even to 11 mantissa
    bits, low 12 bits zeroed. Matches HW DVE rounding (micro-verified)."""
    xi = np.ascontiguousarray(a, np.float32).view(np.uint32)
    lsb = np.uint32(1) << 12
    bias = (lsb >> 1) - 1 + ((xi >> 12) & 1)
    return ((xi + bias) & ~np.uint32(lsb - 1)).view(np.float32)


def make_in_maps(x, W_real, W_imag):
    xs = np.asarray(x, dtype=np.float32).reshape(B_FULL, T)
    Wr = np.asarray(W_real, np.float32)
    Wi = np.asarray(W_imag, np.float32)
    # folded bases, rows j=1..256; j=256 halved (cos) / zero (sin, exact)
    wr_dev = np.zeros((KOUT, KOUT), np.float32)
    wi_dev = np.zeros((KOUT, KOUT), np.float32)
    wr_dev[:255] = Wr[:KOUT, 1:256].T
    wr_dev[255] = 0.5 * Wr[:KOUT, 256]
    wi_dev[:255] = Wi[:KOUT, 1:256].T
    wi_dev[255] = 0.0
    if MM_DT == F32R:
        wr_dev = _round_f32r(wr_dev)
        wi_dev = _round_f32r(wi_dev)
    ident = np.eye(128, dtype=np.float32)
    return [
        {"x": np.ascontiguousarray(xs[i * BPC:(i + 1) * BPC]),
         "wr": wr_dev, "wi": wi_dev, "ident": ident}
        for i in range(N_CORES)
    ]


_NC_CACHE = {}


def kernel(x, W_real, W_imag):
    key = (str(MM_DT), 1)
    if key not in _NC_CACHE:
        _NC_CACHE[key] = build_nc(MM_DT, 1)
    nc = _NC_CACHE[key]
    in_maps = make_in_maps(x, W_real, W_imag)
    res = run_bass_kernel_spmd(nc, in_maps, core_ids=list(range(N_CORES)))
    out = np.concatenate([r["out"] for r in res.results], axis=0)  # [32, F, K]
    out = np.ascontiguousarray(out.transpose(0, 2, 1))             # [32, K, F]
    return out.reshape(B_FULL, C_FULL, KOUT, F).astype(np.float32)



# revision 10
# speedup vs baseline: 1.1784x; 1.1430x over previous
"""DFT spectrogram (nn_DftSpectrogram) Bass kernel for 8 Trainium2 NeuronCores.

Pure data parallel: 32 batch items -> 4 per core. Per item (T=96512 samples):
  - 601 frames of 512 taps (stride 160) are loaded as 5 overlapping blocks of
    128 frames in [frame, tap] layout (contiguous 2KB rows -> efficient DMA)
  - folded with the DFT basis symmetry about tap 256: u[j]=x[j]+x[512-j]
    (cos side), v[j]=x[j]-x[512-j] (sin side) halve the matmul contraction to
    256; the j=0 term is folded into the ACT Square bias; the j=256 basis row
    is halved on the host to absorb the self-pairing
  - u/v are PE-transposed to [tap, frame] and matmul'd against the folded
    cos/sin bases in f32r (fp32 storage, 11-bit-mantissa PE datapath: 4x the
    fp32 matmul rate; measured l2 err ~8e-4 vs the 2e-2 gate). All PE operand
    tiles are f32r-typed so the BIR verifier sees rounding producers; the
    host pre-rounds the bases.
  - log-magnitude + per-frame mean/std normalization over the 256 freqs is
    done in [frame, k] layout: bn_stats/bn_aggr give mean+var in one DVE pass
  - normalized [frame, k] tiles are DMA'd straight out; the host fixes the
    layout to [k, frame] while gathering the 8 core shards

DMA instruction count is minimized (11 per invocation): each dma_start costs
~0.6-1.4us of launching-sequencer + shared-HWDGE time, which dominated HW
time at 31 DMAs. Inputs ride 3D/4D access patterns (item/block/frame/tap),
constants are host-packed into one [128, 1152] tensor, block-4 outputs are
collected in one tile and leave in one strided DMA.

1/(sqrt(var)+eps') is computed entirely on DVE (int bit-trick seed + two
Heron steps) so ACT only ever needs one table set. eps compensation keeps
the algebra exact: (fft-mean)/(std+1e-7) == (g-mean_g)/(std_g+2*ln(10)*1e-7)
for fft = g * 0.5/ln(10).
"""
from contextlib import ExitStack

import numpy as np

import concourse.bass as bass
import concourse.tile as tile
from concourse import bacc, mybir
from concourse.bass_utils import run_bass_kernel_spmd

N_CORES = 8
B_FULL = 32
C_FULL = 1
T = 96512
NFFT = 512
KOUT = 256          # only lower half of the spectrum is kept
SHIFT = 160
F = (T - NFFT) // SHIFT + 1  # 601
BPC = B_FULL // N_CORES      # 4 items per core
EPS = 1e-7
CEPS = float(2.0 * np.log(10.0) * 1e-7)
F0S = (0, 128, 256, 384, 473)  # frame-block starts; last block overlaps by 39
FP32 = mybir.dt.float32
F32R = mybir.dt.float32r
MM_DT = F32R
ITEMW = 5 * NFFT    # 2560 cols per item in the frames tile
NCONST = 1152       # ident(128) + cos(512) + sin(512)


def _build(ctx: ExitStack, tc: "tile.TileContext", xh, ch, outh, mm_dt,
           reps: int):
    nc = tc.nc
    AP = bass.AP
    AF = mybir.ActivationFunctionType
    MDT = mm_dt

    consts = ctx.enter_context(tc.tile_pool(name="consts", bufs=1))
    fpool = ctx.enter_context(tc.tile_pool(name="frames", bufs=2))
    ftpool = ctx.enter_context(tc.tile_pool(name="framesT", bufs=4))
    mpool = ctx.enter_context(tc.tile_pool(name="mag", bufs=8))
    glpool = ctx.enter_context(tc.tile_pool(name="gl", bufs=4))
    spool = ctx.enter_context(tc.tile_pool(name="stats", bufs=6))
    gnpool = ctx.enter_context(tc.tile_pool(name="gnorm", bufs=4))
    ptrp = ctx.enter_context(tc.tile_pool(name="ptr", bufs=3, space="PSUM"))
    prip = ctx.enter_context(tc.tile_pool(name="pri", bufs=5, space="PSUM"))

    call = consts.tile([128, NCONST], MDT, tag="call")
    ident = call[:, 0:128]
    c_sb = call[:, 128:640]
    s_sb = call[:, 640:1152]
    epsb = consts.tile([128, 1], FP32, tag="epsb")
    nc.vector.memset(epsb[:], EPS)

    def stage_front(b, ftile, fb, gl, mv3):
        """u/v fold, transposes, matmuls, squares, ln, bn stats for one block."""
        src_f = ftile[:, b * ITEMW + fb * NFFT:b * ITEMW + (fb + 1) * NFFT]
        # fold: u[j] = x[j] + x[512-j], v[j] = x[j] - x[512-j], j=1..256
        fwd = src_f[:, 1:257]
        rev = src_f[:, 511:255:-1]
        u = mpool.tile([128, KOUT], MDT, tag="u", name="u")
        nc.vector.tensor_add(u[:], fwd, rev)
        v = mpool.tile([128, KOUT], MDT, tag="v", name="v")
        nc.gpsimd.tensor_sub(v[:], fwd, rev)

        # transpose u,v [128f, 256j] -> 4x [128j, 128f] in one PSUM bank
        ptr = ptrp.tile([128, NFFT], MDT, tag="ptr", name="ptr")
        for c, srcc in enumerate((u[:, 0:128], u[:, 128:256],
                                  v[:, 0:128], v[:, 128:256])):
            nc.tensor.matmul(ptr[:, c * 128:(c + 1) * 128],
                             srcc, ident, is_transpose=True,
                             start=(c == 0), stop=(c == 3))
        ft_sb = ftpool.tile([128, NFFT], MDT, tag="ft_sb", name="ft_sb")
        nc.scalar.copy(ft_sb[:, 0:KOUT], ptr[:, 0:KOUT])
        nc.vector.tensor_copy(ft_sb[:, KOUT:2 * KOUT], ptr[:, KOUT:2 * KOUT])

        # real[f,k] = sum_j u[f,j] C[j,k] (+ x[160f], via Square bias)
        # imag[f,k] = sum_j v[f,j] S[j,k]
        pri = prip.tile([128, 2 * KOUT], FP32, tag="pri", name="pri")
        nc.tensor.matmul(pri[:, 0:KOUT], ft_sb[:, 0:128],
                         c_sb[:, 0:KOUT], start=True, stop=False)
        nc.tensor.matmul(pri[:, 0:KOUT], ft_sb[:, 128:256],
                         c_sb[:, KOUT:2 * KOUT], start=False, stop=False)
        nc.tensor.matmul(pri[:, KOUT:2 * KOUT], ft_sb[:, 256:384],
                         s_sb[:, 0:KOUT], start=False, stop=False)
        nc.tensor.matmul(pri[:, KOUT:2 * KOUT], ft_sb[:, 384:512],
                         s_sb[:, KOUT:2 * KOUT], start=False, stop=True)

        sq = mpool.tile([128, 2 * KOUT], FP32, tag="sq", name="sq")
        nc.scalar.activation(sq[:, 0:KOUT], pri[:, 0:KOUT], AF.Square,
                             bias=src_f[:, 0:1])
        nc.scalar.activation(sq[:, KOUT:2 * KOUT], pri[:, KOUT:2 * KOUT],
                             AF.Square)
        msum = mpool.tile([128, KOUT], FP32, tag="msum", name="msum")
        nc.gpsimd.tensor_add(msum[:], sq[:, 0:KOUT], sq[:, KOUT:2 * KOUT])
        gls = gl[:, fb * KOUT:(fb + 1) * KOUT]
        nc.scalar.activation(gls, msum[:], AF.Ln, bias=epsb[:])
        bn6 = spool.tile([128, 6], FP32, tag="bn6", name="bn6")
        nc.vector.bn_stats(bn6[:], gls)
        nc.vector.bn_aggr(mv3[:, fb, :], bn6[:])

    def stage_back(b, gl, mv, gtail, fb_lo=0, fb_hi=4):
        """rden = 1/(sqrt(var)+ceps) on DVE only (int bit-trick sqrt seed +
        two Heron steps, 5e-7 rel; keeps ACT on one table set), then
        normalize and DMA out, for blocks fb_lo..fb_hi of one item."""
        w = fb_hi - fb_lo + 1
        var = bass.AP(mv[:].tensor, mv[:].offset + 2 * fb_lo + 1,
                      [list(mv[:].ap[0]), [2, w]])
        sh = spool.tile([128, w], mybir.dt.int32, tag="sh", name="sh")
        nc.vector.tensor_scalar(sh[:], var.bitcast(mybir.dt.int32), 1, None,
                                op0=mybir.AluOpType.arith_shift_right)
        s0i = spool.tile([128, w], mybir.dt.int32, tag="s0i", name="s0i")
        nc.vector.tensor_scalar(s0i[:], sh[:], 0x1FBD1DF5, None,
                                op0=mybir.AluOpType.add)
        s_cur = s0i[:].bitcast(FP32)
        for it in range(2):
            hr = spool.tile([128, w], FP32, tag=f"hr{it}", name=f"hr{it}")
            nc.vector.reciprocal(hr[:], s_cur)
            ht = spool.tile([128, w], FP32, tag=f"ht{it}", name=f"ht{it}")
            nc.vector.tensor_mul(ht[:], var, hr[:])
            hs = spool.tile([128, w], FP32, tag=f"hs{it}", name=f"hs{it}")
            nc.vector.tensor_add(hs[:], s_cur, ht[:])
            hh = spool.tile([128, w], FP32, tag=f"hh{it}", name=f"hh{it}")
            nc.vector.tensor_scalar_mul(hh[:], hs[:], 0.5)
            s_cur = hh[:]
        uu = spool.tile([128, w], FP32, tag="uu", name="uu")
        nc.vector.tensor_scalar(uu[:], s_cur, 1.0, CEPS,
                                op0=mybir.AluOpType.mult,
                                op1=mybir.AluOpType.add)
        rden = spool.tile([128, w], FP32, tag="rden", name="rden")
        nc.vector.reciprocal(rden[:], uu[:])

        # normalize into one tile so blocks 0..3 leave in a single strided
        # DMA; block 4 (89 new frames) goes into the shared gtail tile and
        # leaves in one merged DMA after the last item
        gn4 = None
        for fb in range(fb_lo, fb_hi + 1):
            gls = gl[:, fb * KOUT:(fb + 1) * KOUT]
            if fb < 4:
                if gn4 is None:
                    gn4 = gnpool.tile([128, 4 * KOUT], FP32, tag="gn4",
                                      name="gn4")
                gdst = gn4[:, fb * KOUT:(fb + 1) * KOUT]
            else:
                gdst = gtail[:, b * KOUT:(b + 1) * KOUT]
            nc.vector.tensor_scalar(gdst, gls,
                                    mv[:, 2 * fb:2 * fb + 1],
                                    rden[:, fb - fb_lo:fb - fb_lo + 1],
                                    op0=mybir.AluOpType.subtract,
                                    op1=mybir.AluOpType.mult)
        if gn4 is not None:
            nblk = min(fb_hi, 3) - fb_lo + 1
            dst = bass.AP(outh, b * F * KOUT + fb_lo * 128 * KOUT,
                          [[KOUT, 128], [128 * KOUT, nblk], [1, KOUT]])
            nc.scalar.dma_start(
                dst, gn4[:, fb_lo * KOUT:(fb_lo + nblk) * KOUT].rearrange(
                    "p (f k) -> p f k", k=KOUT))

    def body():
        # one frames tile holds all items: [p, item(4) x block(5) x tap(512)]
        ftile = fpool.tile([128, BPC * ITEMW], FP32, tag="ftile",
                           name="ftile")
        gtail = gnpool.tile([128, BPC * KOUT], FP32, tag="gtail",
                            name="gtail")
        fbase = ftile[:]
        prow = list(fbase.ap[0])  # [partition pitch, 128]

        def fdst(off, dims):
            return AP(fbase.tensor, fbase.offset + off, [prow] + dims)

        # item 0 block 0 first (gates the pipeline start), then the rest
        nc.sync.dma_start(
            fdst(0, [[1, NFFT]]),
            AP(xh, 0, [[SHIFT, 128], [1, NFFT]]))
        if reps == 1:
            nc.sync.dma_start(call[:], ch.ap().bitcast(MDT))
        nc.sync.dma_start(
            fdst(NFFT, [[NFFT, 3], [1, NFFT]]),
            AP(xh, SHIFT * 128, [[SHIFT, 128], [SHIFT * 128, 3], [1, NFFT]]))
        # items 1-3 blocks 0-3 (3D, one DMA per item — APs cap at 3 dims)
        for b in range(1, BPC):
            nc.sync.dma_start(
                fdst(b * ITEMW, [[NFFT, 4], [1, NFFT]]),
                AP(xh, b * T, [[SHIFT, 128], [SHIFT * 128, 4], [1, NFFT]]))
        # all items' block 4 in one DMA
        nc.sync.dma_start(
            fdst(4 * NFFT, [[ITEMW, BPC], [1, NFFT]]),
            AP(xh, SHIFT * F0S[4], [[SHIFT, 128], [T, BPC], [1, NFFT]]))

        pending = None
        for b in range(BPC):
            last = b == BPC - 1
            gl = glpool.tile([128, 5 * KOUT], FP32, tag="gl", name="gl")
            mv = spool.tile([128, 10], FP32, tag="mv", name="mv")
            mv3 = mv[:].rearrange("p (f two) -> p f two", two=2)
            for fb in range(5):
                stage_front(b, ftile, fb, gl, mv3)
                if fb == 3 and pending is not None:
                    stage_back(*pending)
                    pending = None
                if last and fb == 3:
                    stage_back(b, gl, mv, gtail, 0, 3)
            if not last:
                pending = (b, gl, mv, gtail)
            else:
                stage_back(b, gl, mv, gtail, 4, 4)

        # merged block-4 output: out[b, 512:601, :] <- gtail[39:128, b, :]
        nc.scalar.dma_start(
            AP(outh, 512 * KOUT, [[KOUT, 89], [F * KOUT, BPC], [1, KOUT]]),
            AP(gtail[:].tensor, gtail[:].offset + 39 * gtail[:].ap[0][0],
               [[gtail[:].ap[0][0], 89], [KOUT, BPC], [1, KOUT]]))

    if reps == 1:
        body()
    elif reps < 0:
        # straight-line -reps bodies (no For_i): cost-model steady-state probe
        nc.sync.dma_start(call[:], ch.ap().bitcast(MDT))
        for _ in range(-reps):
            body()
    else:
        # 2 bodies per iteration: tile pools rotate across the pair, so rep
        # N+1's input DMAs land in the alternate frames buffer and overlap
        # rep N's compute (steady-state pipelining across the loop edge)
        assert reps % 2 == 0
        nc.sync.dma_start(call[:], ch.ap().bitcast(MDT))
        with tc.For_i(0, reps // 2, 1):
            body()
            body()


def build_nc(mm_dt=MM_DT, reps: int = 1):
    nc = bacc.Bacc("TRN2", target_bir_lowering=False, debug=False)
    xh = nc.dram_tensor("x", [BPC, T], FP32, kind="ExternalInput")
    ch = nc.dram_tensor("consts", [128, NCONST], FP32, kind="ExternalInput")
    outh = nc.dram_tensor("out", [BPC, F, KOUT], FP32, kind="ExternalOutput")
    with tile.TileContext(nc) as tc, ExitStack() as ctx:
        _build(ctx, tc, xh, ch, outh, mm_dt, reps)
    nc.compile()
    return nc


def _round_f32r(a):
    """Round fp32 to f32r (1-8-11): round-to-nearest-even to 11 mantissa
    bits, low 12 bits zeroed. Matches HW DVE rounding (micro-verified)."""
    xi = np.ascontiguousarray(a, np.float32).view(np.uint32)
    lsb = np.uint32(1) << 12
    bias = (lsb >> 1) - 1 + ((xi >> 12) & 1)
    return ((xi + bias) & ~np.uint32(lsb - 1)).view(np.float32)


def make_in_maps(x, W_real, W_imag):
    xs = np.asarray(x, dtype=np.float32).reshape(B_FULL, T)
    Wr = np.asarray(W_real, np.float32)
    Wi = np.asarray(W_imag, np.float32)
    # folded bases, rows j=1..256; j=256 halved (cos) / zero (sin, exact)
    wr_dev = np.zeros((KOUT, KOUT), np.float32)
    wi_dev = np.zeros((KOUT, KOUT), np.float32)
    wr_dev[:255] = Wr[:KOUT, 1:256].T
    wr_dev[255] = 0.5 * Wr[:KOUT, 256]
    wi_dev[:255] = Wi[:KOUT, 1:256].T
    wi_dev[255] = 0.0
    if MM_DT == F32R:
        wr_dev = _round_f32r(wr_dev)
        wi_dev = _round_f32r(wi_dev)
    # pack [ident | cos(c,k) | sin(c,k)] as one [128, 1152] tensor where
    # chunk c of the 256-row basis maps to rows c*128+p
    consts = np.zeros((128, NCONST), np.float32)
    consts[:, 0:128] = np.eye(128, dtype=np.float32)
    consts[:, 128:640] = wr_dev.reshape(2, 128, KOUT).transpose(
        1, 0, 2).reshape(128, 512)
    consts[:, 640:1152] = wi_dev.reshape(2, 128, KOUT).transpose(
        1, 0, 2).reshape(128, 512)
    return [
        {"x": np.ascontiguousarray(xs[i * BPC:(i + 1) * BPC]),
         "consts": consts}
        for i in range(N_CORES)
    ]


_NC_CACHE = {}


def kernel(x, W_real, W_imag):
    key = (str(MM_DT), 1)
    if key not in _NC_CACHE:
        _NC_CACHE[key] = build_nc(MM_DT, 1)
    nc = _NC_CACHE[key]
    in_maps = make_in_maps(x, W_real, W_imag)
    res = run_bass_kernel_spmd(nc, in_maps, core_ids=list(range(N_CORES)))
    out = np.concatenate([r["out"] for r in res.results], axis=0)  # [32, F, K]
    out = np.ascontiguousarray(out.transpose(0, 2, 1))             # [32, K, F]
    return out.reshape(B_FULL, C_FULL, KOUT, F).astype(np.float32)


# revision 25
# speedup vs baseline: 1.6607x; 1.4093x over previous
"""DFT spectrogram (nn_DftSpectrogram) Bass kernel for 8 Trainium2 NeuronCores.

Pure data parallel: 32 batch items -> 4 per core. Per item (T=96512 samples):
601 frames x 512 taps (stride 160) -> DFT -> log10|X| (lower 256 bins) ->
per-frame mean/std normalization over frequency.

Architecture (HW-profiled on TRN2; per-instruction overhead and cross-engine
semaphore latency dominate, so the design minimizes instruction count and
in-order-engine stalls):
  - frames land in [frame, tap] layout via 6 strided DMAs (2KB rows); all
    PE operands are float32r-typed (fp32 storage, 11-bit-mantissa PE
    datapath, 1 cycle/row vs fp32's 4 at moving-dim >= 256; l2 err ~1e-3
    vs the 2e-2 gate)
  - no tap-fold: at the f32r rate, folding costs more in DVE/Pool fold ops
    and the j=0 bias square than the halved contraction saves on PE
  - units of 2 adjacent 128-frame blocks are software-pipelined: PE
    transposes run one unit AHEAD of the unit's matmuls (the in-order PE
    stream never waits on the PSUM->SBUF copies), ln/bn_stats lag one unit
    BEHIND (the in-order ACT stream never waits on the Pool mag-sum)
  - wide ops via 2D/3D access patterns: one Square per unit [128,1024],
    one Ln [128,512], one Pool mag-sum, copies split ACT/DVE
  - per item, the normalized output leaves in ONE DMA of 128 contiguous
    5KB runs (HBM layout [b][p][blk][k]; host reassembles frame order) -
    HW DMA cost is descriptor-rate-bound, not byte-bound
  - 1/(sqrt(var)+eps') on DVE only (int bit-trick seed + 2 Heron steps) so
    ACT stays on one activation-table set; eps compensation keeps the
    algebra exact: (fft-mean)/(std+1e-7) == (g-mean_g)/(std_g+2*ln10*1e-7)
    for fft = g * 0.5/ln10
  - the reps>1 timing variant unrolls 2 bodies per For_i iteration so tile
    pools double-buffer across the loop edge (input DMAs of rep N+1 overlap
    rep N's compute)
"""
from contextlib import ExitStack

import numpy as np

import concourse.bass as bass
import concourse.tile as tile
from concourse import bacc, mybir
from concourse.bass_utils import run_bass_kernel_spmd

N_CORES = 8
B_FULL = 32
C_FULL = 1
T = 96512
NFFT = 512
KOUT = 256          # only lower half of the spectrum is kept
SHIFT = 160
F = (T - NFFT) // SHIFT + 1  # 601
BPC = B_FULL // N_CORES      # 4 items per core
EPS = 1e-7
CEPS = float(2.0 * np.log(10.0) * 1e-7)
F0S = (0, 128, 256, 384, 473)  # frame-block starts; last block overlaps by 39
FP32 = mybir.dt.float32
F32R = mybir.dt.float32r
MM_DT = F32R
ITEMW = 5 * NFFT    # 2560 cols per item in the frames tile
NCONST = 2176       # ident(128) + cos(4x256) + sin(4x256)


def _build(ctx: ExitStack, tc: "tile.TileContext", xh, ch, outh, mm_dt,
           reps: int, variant: str = "full"):
    nc = tc.nc
    AP = bass.AP
    AF = mybir.ActivationFunctionType
    MDT = mm_dt

    consts = ctx.enter_context(tc.tile_pool(name="consts", bufs=1))
    fpool = ctx.enter_context(tc.tile_pool(name="frames", bufs=2))
    ftpool = ctx.enter_context(tc.tile_pool(name="framesT", bufs=4))
    mpool = ctx.enter_context(tc.tile_pool(name="mag", bufs=6))
    glpool = ctx.enter_context(tc.tile_pool(name="gl", bufs=3))
    spool = ctx.enter_context(tc.tile_pool(name="stats", bufs=6))
    gnpool = ctx.enter_context(tc.tile_pool(name="gnorm", bufs=3))
    ptrp = ctx.enter_context(tc.tile_pool(name="ptr", bufs=2, space="PSUM"))
    prip = ctx.enter_context(tc.tile_pool(name="pri", bufs=2, space="PSUM"))

    call = consts.tile([128, NCONST], MDT, tag="call")
    ident = call[:, 0:128]
    c_sb = call[:, 128:1152]
    s_sb = call[:, 1152:2176]
    epsb = consts.tile([128, 1], FP32, tag="epsb")
    nc.vector.memset(epsb[:], EPS)

    def sap(base, off, dims):
        """Raw strided AP into a tile: [partition row] + free dims."""
        return AP(base.tensor, base.offset + off, [list(base.ap[0])] + dims)

    def stage_tr(ftile, b, fb0, nb):
        """PE transposes for one unit (nb adjacent 128-frame blocks).

        Emitted one unit AHEAD of the unit's matmuls so the in-order PE
        stream never stalls waiting for the PSUM->SBUF copies: while the
        copies of unit i drain, the PE transposes unit i+1."""
        fbase = ftile[:]
        soff = b * ITEMW + fb0 * NFFT
        ptr = ptrp.tile([128, 2 * NFFT], MDT, tag="ptr", name="ptr")
        for h in range(nb):
            o = soff + h * NFFT
            for c in range(4):
                nc.tensor.matmul(ptr[:, h * NFFT + c * 128:
                                     h * NFFT + (c + 1) * 128],
                                 fbase[:, o + c * 128:o + (c + 1) * 128],
                                 ident, is_transpose=True,
                                 start=(c == 0), stop=(c == 3))
        return ptr

    def stage_mid(ptr, b, fb0, nb, gl):
        """copies, DFT matmuls, square, mag-sum for one unit."""
        half = nb * KOUT
        ft_sb = ftpool.tile([128, 2 * NFFT], MDT, tag="ft_sb", name="ft_sb")
        nc.scalar.copy(ft_sb[:, 0:half], ptr[:, 0:half])
        nc.vector.tensor_copy(ft_sb[:, half:2 * half], ptr[:, half:2 * half])
        if variant == "lad1":
            return None

        # real[f,k] = sum_t x[f,t] C[t,k]; imag[f,k] = sum_t x[f,t] S[t,k]
        pri = prip.tile([128, 2 * NFFT], FP32, tag="pri", name="pri")
        for h in range(nb):
            o = h * NFFT
            for q in range(4):
                nc.tensor.matmul(pri[:, o:o + KOUT],
                                 ft_sb[:, o + q * 128:o + (q + 1) * 128],
                                 c_sb[:, q * KOUT:(q + 1) * KOUT],
                                 start=(q == 0), stop=(q == 3))
            for q in range(4):
                nc.tensor.matmul(pri[:, o + KOUT:o + 2 * KOUT],
                                 ft_sb[:, o + q * 128:o + (q + 1) * 128],
                                 s_sb[:, q * KOUT:(q + 1) * KOUT],
                                 start=(q == 0), stop=(q == 3))
        if variant == "lad2":
            return None

        sq = mpool.tile([128, 2 * NFFT], FP32, tag="sq", name="sq")
        nc.scalar.activation(sq[:, 0:nb * NFFT], pri[:, 0:nb * NFFT],
                             AF.Square)
        msum = mpool.tile([128, 2 * KOUT], FP32, tag="msum", name="msum")
        nc.gpsimd.tensor_add(
            msum[:, 0:nb * KOUT].rearrange("p (n k) -> p n k", k=KOUT),
            sap(sq[:], 0, [[NFFT, nb], [1, KOUT]]),
            sap(sq[:], KOUT, [[NFFT, nb], [1, KOUT]]))
        return msum

    def stage_tail(msum, b, fb0, nb, gl, mv3):
        """ln + bn stats for one unit; lags one slot behind stage_mid so the
        in-order ACT stream never parks on the Pool mag-sum."""
        if msum is None:
            return
        gls = gl[:, fb0 * KOUT:(fb0 + nb) * KOUT]
        nc.scalar.activation(gls, msum[:, 0:nb * KOUT], AF.Ln, bias=epsb[:])
        if variant == "lad3":
            return
        for h in range(nb):
            bn6 = spool.tile([128, 6], FP32, tag="bn6", name="bn6")
            nc.vector.bn_stats(bn6[:], gl[:, (fb0 + h) * KOUT:
                                           (fb0 + h + 1) * KOUT])
            nc.vector.bn_aggr(mv3[:, fb0 + h, :], bn6[:])

    def stage_back(b, gl, mv, gn5, fb_lo=0, fb_hi=4):
        """rden = 1/(sqrt(var)+ceps) on DVE only (int bit-trick sqrt seed +
        two Heron steps, 5e-7 rel; keeps ACT on one table set), then
        normalize and DMA out, for blocks fb_lo..fb_hi of one item."""
        w = fb_hi - fb_lo + 1
        var = bass.AP(mv[:].tensor, mv[:].offset + 2 * fb_lo + 1,
                      [list(mv[:].ap[0]), [2, w]])
        sh = spool.tile([128, w], mybir.dt.int32, tag="sh", name="sh")
        nc.vector.tensor_scalar(sh[:], var.bitcast(mybir.dt.int32), 1, None,
                                op0=mybir.AluOpType.arith_shift_right)
        s0i = spool.tile([128, w], mybir.dt.int32, tag="s0i", name="s0i")
        nc.vector.tensor_scalar(s0i[:], sh[:], 0x1FBD1DF5, None,
                                op0=mybir.AluOpType.add)
        s_cur = s0i[:].bitcast(FP32)
        for it in range(2):
            hr = spool.tile([128, w], FP32, tag=f"hr{it}", name=f"hr{it}")
            nc.vector.reciprocal(hr[:], s_cur)
            ht = spool.tile([128, w], FP32, tag=f"ht{it}", name=f"ht{it}")
            nc.vector.tensor_mul(ht[:], var, hr[:])
            hs = spool.tile([128, w], FP32, tag=f"hs{it}", name=f"hs{it}")
            nc.vector.tensor_add(hs[:], s_cur, ht[:])
            hh = spool.tile([128, w], FP32, tag=f"hh{it}", name=f"hh{it}")
            nc.vector.tensor_scalar_mul(hh[:], hs[:], 0.5)
            s_cur = hh[:]
        uu = spool.tile([128, w], FP32, tag="uu", name="uu")
        nc.vector.tensor_scalar(uu[:], s_cur, 1.0, CEPS,
                                op0=mybir.AluOpType.mult,
                                op1=mybir.AluOpType.add)
        rden = spool.tile([128, w], FP32, tag="rden", name="rden")
        nc.vector.reciprocal(rden[:], uu[:])

        # normalize into gn5 [p, blk, k]; the whole item leaves in ONE DMA
        # of 128 contiguous 5KB runs (out HBM layout is [b][p][blk][k];
        # the host reassembles frame order for free)
        for fb in range(fb_lo, fb_hi + 1):
            gls = gl[:, fb * KOUT:(fb + 1) * KOUT]
            nc.vector.tensor_scalar(gn5[:, fb * KOUT:(fb + 1) * KOUT], gls,
                                    mv[:, 2 * fb:2 * fb + 1],
                                    rden[:, fb - fb_lo:fb - fb_lo + 1],
                                    op0=mybir.AluOpType.subtract,
                                    op1=mybir.AluOpType.mult)
        if fb_hi == 4 and variant != "compute":
            nc.scalar.dma_start(
                bass.AP(outh, b * 128 * 5 * KOUT,
                        [[5 * KOUT, 128], [1, 5 * KOUT]]),
                gn5[:])

    def body(variant=variant):
        # one frames tile holds all items: [p, item(4) x block(5) x tap(512)]
        ftile = fpool.tile([128, BPC * ITEMW], MDT, tag="ftile",
                           name="ftile")
        if variant == "compute":
            nc.vector.memset(ftile[:, 0:1], 0.0)
        fbase = ftile[:]
        prow = list(fbase.ap[0])  # [partition pitch, 128]

        def fdst(off, dims):
            return AP(fbase.tensor, fbase.offset + off, [prow] + dims)

        # item 0 block 0 first (gates the pipeline start), then the rest
        if variant != "compute":
          nc.sync.dma_start(
            fdst(0, [[1, NFFT]]),
            AP(xh, 0, [[SHIFT, 128], [1, NFFT]]).bitcast(MDT))
        if reps == 1:
            nc.sync.dma_start(call[:], ch.ap().bitcast(MDT))
        if variant != "compute":
          nc.sync.dma_start(
            fdst(NFFT, [[NFFT, 3], [1, NFFT]]),
            AP(xh, SHIFT * 128, [[SHIFT, 128], [SHIFT * 128, 3], [1, NFFT]]).bitcast(MDT))
          # items 1-3 blocks 0-3 (3D, one DMA per item — APs cap at 3 dims)
          for b in range(1, BPC):
            nc.sync.dma_start(
                fdst(b * ITEMW, [[NFFT, 4], [1, NFFT]]),
                AP(xh, b * T, [[SHIFT, 128], [SHIFT * 128, 4], [1, NFFT]]).bitcast(MDT))
          # all items' block 4 in one DMA
          nc.sync.dma_start(
            fdst(4 * NFFT, [[ITEMW, BPC], [1, NFFT]]),
            AP(xh, SHIFT * F0S[4], [[SHIFT, 128], [T, BPC], [1, NFFT]]).bitcast(MDT))

        if variant == "dma":
            for b in range(BPC):
                gn5 = gnpool.tile([128, 5 * KOUT], FP32, tag="gn5",
                                  name="gn5")
                nc.vector.memset(gn5[:, 0:1], 0.0)
                nc.scalar.dma_start(
                    bass.AP(outh, b * 128 * 5 * KOUT,
                            [[5 * KOUT, 128], [1, 5 * KOUT]]),
                    gn5[:])
            return

        # software pipeline over 12 units (4 items x (2 pairs + 1 single)):
        # PE transposes run one unit ahead; ln/bn lag one unit behind
        lad = variant.startswith("lad")
        units = [(b, fb0, nb) for b in range(BPC)
                 for fb0, nb in ((0, 2), (2, 2), (4, 1))]
        item = {}
        for b in range(BPC):
            gl = glpool.tile([128, 5 * KOUT], FP32, tag="gl", name="gl")
            gn5 = gnpool.tile([128, 5 * KOUT], FP32, tag="gn5", name="gn5")
            mv = spool.tile([128, 10], FP32, tag="mv", name="mv")
            item[b] = (gl, gn5, mv,
                       mv[:].rearrange("p (f two) -> p f two", two=2))

        ptrs = [None] * len(units)
        msums = [None] * len(units)
        ptrs[0] = stage_tr(ftile, *units[0])
        for i, (b, fb0, nb) in enumerate(units):
            gl, gn5, mv, mv3 = item[b]
            if i + 1 < len(units):
                ptrs[i + 1] = stage_tr(ftile, *units[i + 1])
            if i >= 1:
                pb, pf, pn = units[i - 1]
                stage_tail(msums[i - 1], pb, pf, pn, item[pb][0], item[pb][3])
                if not lad:
                    if pf == 4 and pb < BPC - 1:
                        # item pb fully binned: normalize + output it
                        stage_back(pb, item[pb][0], item[pb][2],
                                   item[pb][1], 0, 4)
                    if pb == BPC - 1 and pf == 2:
                        stage_back(pb, gl, mv, gn5, 0, 3)
            msums[i] = stage_mid(ptrs[i], b, fb0, nb, gl)
        lb, lf, ln_ = units[-1]
        stage_tail(msums[-1], lb, lf, ln_, item[lb][0], item[lb][3])
        if not lad:
            stage_back(lb, item[lb][0], item[lb][2], item[lb][1], 4, 4)
        else:
            for b in range(BPC):
                gn5 = item[b][1]
                nc.vector.memset(gn5[:, 0:1], 0.0)
                nc.scalar.dma_start(
                    bass.AP(outh, b * 128 * 5 * KOUT,
                            [[5 * KOUT, 128], [1, 5 * KOUT]]),
                    gn5[:])


    if reps == 1:
        body()
    elif reps < 0:
        # straight-line -reps bodies (no For_i): cost-model steady-state probe
        nc.sync.dma_start(call[:], ch.ap().bitcast(MDT))
        for _ in range(-reps):
            body()
    else:
        # 2 bodies per iteration: tile pools rotate across the pair, so rep
        # N+1's input DMAs land in the alternate frames buffer and overlap
        # rep N's compute (steady-state pipelining across the loop edge)
        assert reps % 2 == 0
        nc.sync.dma_start(call[:], ch.ap().bitcast(MDT))
        with tc.For_i(0, reps // 2, 1):
            body()
            body()


def build_nc(mm_dt=MM_DT, reps: int = 1, variant: str = "full"):
    nc = bacc.Bacc("TRN2", target_bir_lowering=False, debug=False)
    xh = nc.dram_tensor("x", [BPC, T], FP32, kind="ExternalInput")
    ch = nc.dram_tensor("consts", [128, NCONST], FP32, kind="ExternalInput")
    outh = nc.dram_tensor("out", [BPC, 128, 5 * KOUT], FP32,
                          kind="ExternalOutput")
    with tile.TileContext(nc) as tc, ExitStack() as ctx:
        _build(ctx, tc, xh, ch, outh, mm_dt, reps, variant)
    nc.compile()
    return nc


def _round_f32r(a):
    """Round fp32 to f32r (1-8-11): round-to-nearest-even to 11 mantissa
    bits, low 12 bits zeroed. Matches HW DVE rounding (micro-verified)."""
    xi = np.ascontiguousarray(a, np.float32).view(np.uint32)
    lsb = np.uint32(1) << 12
    bias = (lsb >> 1) - 1 + ((xi >> 12) & 1)
    return ((xi + bias) & ~np.uint32(lsb - 1)).view(np.float32)


def make_in_maps(x, W_real, W_imag):
    xs = np.asarray(x, dtype=np.float32).reshape(B_FULL, T)
    Wr = np.asarray(W_real, np.float32)
    Wi = np.asarray(W_imag, np.float32)
    # unfolded bases: C[t, k] = Wr[k, t], S[t, k] = Wi[k, t], t = 128q+p
    wr_dev = np.ascontiguousarray(Wr[:KOUT, :].T)   # [512 t, 256 k]
    wi_dev = np.ascontiguousarray(Wi[:KOUT, :].T)
    if MM_DT == F32R:
        wr_dev = _round_f32r(wr_dev)
        wi_dev = _round_f32r(wi_dev)
    # pack [ident | cos(q,k) | sin(q,k)] as one [128, 2176] tensor where
    # chunk q of the 512-row basis maps to rows 128q+p
    consts = np.zeros((128, NCONST), np.float32)
    consts[:, 0:128] = np.eye(128, dtype=np.float32)
    consts[:, 128:1152] = wr_dev.reshape(4, 128, KOUT).transpose(
        1, 0, 2).reshape(128, 1024)
    consts[:, 1152:2176] = wi_dev.reshape(4, 128, KOUT).transpose(
        1, 0, 2).reshape(128, 1024)
    return [
        {"x": np.ascontiguousarray(xs[i * BPC:(i + 1) * BPC]),
         "consts": consts}
        for i in range(N_CORES)
    ]


_NC_CACHE = {}


def kernel(x, W_real, W_imag):
    key = (str(MM_DT), 1)
    if key not in _NC_CACHE:
        _NC_CACHE[key] = build_nc(MM_DT, 1)
    nc = _NC_CACHE[key]
    in_maps = make_in_maps(x, W_real, W_imag)
    res = run_bass_kernel_spmd(nc, in_maps, core_ids=list(range(N_CORES)))
    dev = np.concatenate([r["out"] for r in res.results], axis=0)
    dev = dev.reshape(B_FULL, 128, 5, KOUT)        # [b, p, blk, k]
    full = np.empty((B_FULL, F, KOUT), np.float32)
    blocks = dev.transpose(0, 2, 1, 3)             # [b, blk, p, k]
    full[:, 0:512] = blocks[:, 0:4].reshape(B_FULL, 512, KOUT)
    full[:, 512:601] = blocks[:, 4, 39:128]        # frames 473..600, keep 512+
    out = np.ascontiguousarray(full.transpose(0, 2, 1))  # [32, K, F]
    return out.reshape(B_FULL, C_FULL, KOUT, F).astype(np.float32)


# revision 26
# speedup vs baseline: 1.8381x; 1.1068x over previous
"""DFT spectrogram (nn_DftSpectrogram) Bass kernel for 8 Trainium2 NeuronCores.

Pure data parallel: 32 batch items -> 4 per core. Per item (T=96512 samples):
601 frames x 512 taps (stride 160) -> DFT -> log10|X| (lower 256 bins) ->
per-frame mean/std normalization over frequency.

Architecture (HW-profiled on TRN2; per-instruction overhead and cross-engine
semaphore latency dominate, so the design minimizes instruction count and
in-order-engine stalls):
  - frames land in [frame, tap] layout via 6 strided DMAs (2KB rows); all
    PE operands are float32r-typed (fp32 storage, 11-bit-mantissa PE
    datapath, 1 cycle/row vs fp32's 4 at moving-dim >= 256; l2 err ~1e-3
    vs the 2e-2 gate)
  - no tap-fold: at the f32r rate, folding costs more in DVE/Pool fold ops
    and the j=0 bias square than the halved contraction saves on PE
  - units of 2 adjacent 128-frame blocks are software-pipelined: PE
    transposes run one unit AHEAD of the unit's matmuls (the in-order PE
    stream never waits on the PSUM->SBUF copies), ln/bn_stats lag one unit
    BEHIND (the in-order ACT stream never waits on the Pool mag-sum)
  - wide ops via 2D/3D access patterns: one Square per unit [128,1024],
    one Ln [128,512], one Pool mag-sum, copies split ACT/DVE
  - per item, the normalized output leaves in ONE DMA of 128 contiguous
    5KB runs (HBM layout [b][p][blk][k]; host reassembles frame order) -
    HW DMA cost is descriptor-rate-bound, not byte-bound
  - 1/(sqrt(var)+eps') on DVE only (int bit-trick seed + 2 Heron steps) so
    ACT stays on one activation-table set; eps compensation keeps the
    algebra exact: (fft-mean)/(std+1e-7) == (g-mean_g)/(std_g+2*ln10*1e-7)
    for fft = g * 0.5/ln10
  - the reps>1 timing variant unrolls 2 bodies per For_i iteration so tile
    pools double-buffer across the loop edge (input DMAs of rep N+1 overlap
    rep N's compute)
"""
from contextlib import ExitStack

import numpy as np

import concourse.bass as bass
import concourse.tile as tile
from concourse import bacc, mybir
from concourse.bass_utils import run_bass_kernel_spmd

N_CORES = 8
B_FULL = 32
C_FULL = 1
T = 96512
NFFT = 512
KOUT = 256          # only lower half of the spectrum is kept
SHIFT = 160
F = (T - NFFT) // SHIFT + 1  # 601
BPC = B_FULL // N_CORES      # 4 items per core
EPS = 1e-7
CEPS = float(2.0 * np.log(10.0) * 1e-7)
F0S = (0, 128, 256, 384, 473)  # frame-block starts; last block overlaps by 39
FP32 = mybir.dt.float32
F32R = mybir.dt.float32r
MM_DT = F32R
ITEMW = 5 * NFFT    # 2560 cols per item in the frames tile
NCONST = 2176       # ident(128) + cos(4x256) + sin(4x256)


def _build(ctx: ExitStack, tc: "tile.TileContext", xh, ch, outh, mm_dt,
           reps: int, variant: str = "full"):
    nc = tc.nc
    AP = bass.AP
    AF = mybir.ActivationFunctionType
    MDT = mm_dt

    consts = ctx.enter_context(tc.tile_pool(name="consts", bufs=1))
    fpool = ctx.enter_context(tc.tile_pool(name="frames", bufs=2))
    ftpool = ctx.enter_context(tc.tile_pool(name="framesT", bufs=4))
    mpool = ctx.enter_context(tc.tile_pool(name="mag", bufs=6))
    glpool = ctx.enter_context(tc.tile_pool(name="gl", bufs=4))
    spool = ctx.enter_context(tc.tile_pool(name="stats", bufs=10))
    gnpool = ctx.enter_context(tc.tile_pool(name="gnorm", bufs=4))
    ptrp = ctx.enter_context(tc.tile_pool(name="ptr", bufs=2, space="PSUM"))
    prip = ctx.enter_context(tc.tile_pool(name="pri", bufs=2, space="PSUM"))

    call = consts.tile([128, NCONST], MDT, tag="call")
    ident = call[:, 0:128]
    c_sb = call[:, 128:1152]
    s_sb = call[:, 1152:2176]
    epsb = consts.tile([128, 1], FP32, tag="epsb")
    nc.vector.memset(epsb[:], EPS)

    def sap(base, off, dims):
        """Raw strided AP into a tile: [partition row] + free dims."""
        return AP(base.tensor, base.offset + off, [list(base.ap[0])] + dims)

    def stage_tr(ftile, b, fb0, nb):
        """PE transposes for one unit (nb adjacent 128-frame blocks).

        Emitted one unit AHEAD of the unit's matmuls so the in-order PE
        stream never stalls waiting for the PSUM->SBUF copies: while the
        copies of unit i drain, the PE transposes unit i+1."""
        fbase = ftile[:]
        soff = b * ITEMW + fb0 * NFFT
        ptr = ptrp.tile([128, 2 * NFFT], MDT, tag="ptr", name="ptr")
        for h in range(nb):
            o = soff + h * NFFT
            for c in range(4):
                nc.tensor.matmul(ptr[:, h * NFFT + c * 128:
                                     h * NFFT + (c + 1) * 128],
                                 fbase[:, o + c * 128:o + (c + 1) * 128],
                                 ident, is_transpose=True,
                                 start=(c == 0), stop=(c == 3))
        return ptr

    def stage_mid(ptr, b, fb0, nb, gl):
        """copies, DFT matmuls, square, mag-sum for one unit."""
        half = nb * KOUT
        ft_sb = ftpool.tile([128, 2 * NFFT], MDT, tag="ft_sb", name="ft_sb")
        nc.scalar.copy(ft_sb[:, 0:half], ptr[:, 0:half])
        nc.vector.tensor_copy(ft_sb[:, half:2 * half], ptr[:, half:2 * half])
        if variant == "lad1":
            return None

        # real[f,k] = sum_t x[f,t] C[t,k]; imag[f,k] = sum_t x[f,t] S[t,k]
        pri = prip.tile([128, 2 * NFFT], FP32, tag="pri", name="pri")
        for h in range(nb):
            o = h * NFFT
            for q in range(4):
                nc.tensor.matmul(pri[:, o:o + KOUT],
                                 ft_sb[:, o + q * 128:o + (q + 1) * 128],
                                 c_sb[:, q * KOUT:(q + 1) * KOUT],
                                 start=(q == 0), stop=(q == 3))
            for q in range(4):
                nc.tensor.matmul(pri[:, o + KOUT:o + 2 * KOUT],
                                 ft_sb[:, o + q * 128:o + (q + 1) * 128],
                                 s_sb[:, q * KOUT:(q + 1) * KOUT],
                                 start=(q == 0), stop=(q == 3))
        if variant == "lad2":
            return None

        sq = mpool.tile([128, 2 * NFFT], FP32, tag="sq", name="sq")
        nc.scalar.activation(sq[:, 0:nb * NFFT], pri[:, 0:nb * NFFT],
                             AF.Square)
        msum = mpool.tile([128, 2 * KOUT], FP32, tag="msum", name="msum")
        nc.gpsimd.tensor_add(
            msum[:, 0:nb * KOUT].rearrange("p (n k) -> p n k", k=KOUT),
            sap(sq[:], 0, [[NFFT, nb], [1, KOUT]]),
            sap(sq[:], KOUT, [[NFFT, nb], [1, KOUT]]))
        return msum

    def stage_tail(msum, b, fb0, nb, gl, mv3):
        """ln + bn stats for one unit; lags one slot behind stage_mid so the
        in-order ACT stream never parks on the Pool mag-sum."""
        if msum is None:
            return
        gls = gl[:, fb0 * KOUT:(fb0 + nb) * KOUT]
        nc.scalar.activation(gls, msum[:, 0:nb * KOUT], AF.Ln, bias=epsb[:])
        if variant == "lad3":
            return
        for h in range(nb):
            bn6 = spool.tile([128, 6], FP32, tag="bn6", name="bn6")
            nc.vector.bn_stats(bn6[:], gl[:, (fb0 + h) * KOUT:
                                           (fb0 + h + 1) * KOUT])
            nc.vector.bn_aggr(mv3[:, fb0 + h, :], bn6[:])

    def stage_back(b, gl, mv, gn5, fb_lo=0, fb_hi=4):
        """rden = 1/(sqrt(var)+ceps) on DVE only (int bit-trick sqrt seed +
        two Heron steps, 5e-7 rel; keeps ACT on one table set), then
        normalize and DMA out, for blocks fb_lo..fb_hi of one item."""
        w = fb_hi - fb_lo + 1
        var = bass.AP(mv[:].tensor, mv[:].offset + 2 * fb_lo + 1,
                      [list(mv[:].ap[0]), [2, w]])
        sh = spool.tile([128, w], mybir.dt.int32, tag="sh", name="sh")
        nc.vector.tensor_scalar(sh[:], var.bitcast(mybir.dt.int32), 1, None,
                                op0=mybir.AluOpType.arith_shift_right)
        s0i = spool.tile([128, w], mybir.dt.int32, tag="s0i", name="s0i")
        nc.vector.tensor_scalar(s0i[:], sh[:], 0x1FBD1DF5, None,
                                op0=mybir.AluOpType.add)
        s_cur = s0i[:].bitcast(FP32)
        for it in range(2):
            hr = spool.tile([128, w], FP32, tag=f"hr{it}", name=f"hr{it}")
            nc.vector.reciprocal(hr[:], s_cur)
            ht = spool.tile([128, w], FP32, tag=f"ht{it}", name=f"ht{it}")
            nc.vector.tensor_mul(ht[:], var, hr[:])
            hs = spool.tile([128, w], FP32, tag=f"hs{it}", name=f"hs{it}")
            nc.vector.tensor_add(hs[:], s_cur, ht[:])
            hh = spool.tile([128, w], FP32, tag=f"hh{it}", name=f"hh{it}")
            nc.vector.tensor_scalar_mul(hh[:], hs[:], 0.5)
            s_cur = hh[:]
        uu = spool.tile([128, w], FP32, tag="uu", name="uu")
        nc.vector.tensor_scalar(uu[:], s_cur, 1.0, CEPS,
                                op0=mybir.AluOpType.mult,
                                op1=mybir.AluOpType.add)
        rden = spool.tile([128, w], FP32, tag="rden", name="rden")
        nc.vector.reciprocal(rden[:], uu[:])

        # normalize into gn5 [p, blk, k]; the whole item leaves in ONE DMA
        # of 128 contiguous 5KB runs (out HBM layout is [b][p][blk][k];
        # the host reassembles frame order for free)
        for fb in range(fb_lo, fb_hi + 1):
            gls = gl[:, fb * KOUT:(fb + 1) * KOUT]
            nc.vector.tensor_scalar(gn5[:, fb * KOUT:(fb + 1) * KOUT], gls,
                                    mv[:, 2 * fb:2 * fb + 1],
                                    rden[:, fb - fb_lo:fb - fb_lo + 1],
                                    op0=mybir.AluOpType.subtract,
                                    op1=mybir.AluOpType.mult)
        if fb_hi == 4 and variant != "compute":
            nc.scalar.dma_start(
                bass.AP(outh, b * 128 * 5 * KOUT,
                        [[5 * KOUT, 128], [1, 5 * KOUT]]),
                gn5[:])

    def body(variant=variant):
        # one frames tile holds all items: [p, item(4) x block(5) x tap(512)]
        ftile = fpool.tile([128, BPC * ITEMW], MDT, tag="ftile",
                           name="ftile")
        if variant == "compute":
            nc.vector.memset(ftile[:, 0:1], 0.0)
        fbase = ftile[:]
        prow = list(fbase.ap[0])  # [partition pitch, 128]

        def fdst(off, dims):
            return AP(fbase.tensor, fbase.offset + off, [prow] + dims)

        # item 0 block 0 first (gates the pipeline start), then the rest
        if variant != "compute":
          nc.sync.dma_start(
            fdst(0, [[1, NFFT]]),
            AP(xh, 0, [[SHIFT, 128], [1, NFFT]]).bitcast(MDT))
        if reps == 1:
            nc.sync.dma_start(call[:], ch.ap().bitcast(MDT))
        if variant != "compute":
          nc.sync.dma_start(
            fdst(NFFT, [[NFFT, 3], [1, NFFT]]),
            AP(xh, SHIFT * 128, [[SHIFT, 128], [SHIFT * 128, 3], [1, NFFT]]).bitcast(MDT))
          # items 1-3 blocks 0-3 (3D, one DMA per item — APs cap at 3 dims)
          for b in range(1, BPC):
            nc.sync.dma_start(
                fdst(b * ITEMW, [[NFFT, 4], [1, NFFT]]),
                AP(xh, b * T, [[SHIFT, 128], [SHIFT * 128, 4], [1, NFFT]]).bitcast(MDT))
          # all items' block 4 in one DMA
          nc.sync.dma_start(
            fdst(4 * NFFT, [[ITEMW, BPC], [1, NFFT]]),
            AP(xh, SHIFT * F0S[4], [[SHIFT, 128], [T, BPC], [1, NFFT]]).bitcast(MDT))

        if variant == "dma":
            for b in range(BPC):
                gn5 = gnpool.tile([128, 5 * KOUT], FP32, tag="gn5",
                                  name="gn5")
                nc.vector.memset(gn5[:, 0:1], 0.0)
                nc.scalar.dma_start(
                    bass.AP(outh, b * 128 * 5 * KOUT,
                            [[5 * KOUT, 128], [1, 5 * KOUT]]),
                    gn5[:])
            return

        # software pipeline over 12 units (4 items x (2 pairs + 1 single)):
        # PE transposes run one unit ahead; ln/bn lag one unit behind
        lad = variant.startswith("lad")
        units = [(b, fb0, nb) for b in range(BPC)
                 for fb0, nb in ((0, 2), (2, 2), (4, 1))]
        item = {}
        for b in range(BPC):
            gl = glpool.tile([128, 5 * KOUT], FP32, tag="gl", name="gl")
            gn5 = gnpool.tile([128, 5 * KOUT], FP32, tag="gn5", name="gn5")
            mv = spool.tile([128, 10], FP32, tag="mv", name="mv")
            item[b] = (gl, gn5, mv,
                       mv[:].rearrange("p (f two) -> p f two", two=2))

        ptrs = [None] * len(units)
        msums = [None] * len(units)
        ptrs[0] = stage_tr(ftile, *units[0])
        for i, (b, fb0, nb) in enumerate(units):
            gl, gn5, mv, mv3 = item[b]
            if i + 1 < len(units):
                ptrs[i + 1] = stage_tr(ftile, *units[i + 1])
            if i >= 1:
                pb, pf, pn = units[i - 1]
                stage_tail(msums[i - 1], pb, pf, pn, item[pb][0], item[pb][3])
                if not lad:
                    if pf == 4 and pb < BPC - 1:
                        # item pb fully binned: normalize + output it
                        stage_back(pb, item[pb][0], item[pb][2],
                                   item[pb][1], 0, 4)
                    if pb == BPC - 1 and pf == 2:
                        stage_back(pb, gl, mv, gn5, 0, 3)
            msums[i] = stage_mid(ptrs[i], b, fb0, nb, gl)
        lb, lf, ln_ = units[-1]
        stage_tail(msums[-1], lb, lf, ln_, item[lb][0], item[lb][3])
        if not lad:
            stage_back(lb, item[lb][0], item[lb][2], item[lb][1], 4, 4)
        else:
            for b in range(BPC):
                gn5 = item[b][1]
                nc.vector.memset(gn5[:, 0:1], 0.0)
                nc.scalar.dma_start(
                    bass.AP(outh, b * 128 * 5 * KOUT,
                            [[5 * KOUT, 128], [1, 5 * KOUT]]),
                    gn5[:])


    if reps == 1:
        body()
    elif reps < 0:
        # straight-line -reps bodies (no For_i): cost-model steady-state probe
        nc.sync.dma_start(call[:], ch.ap().bitcast(MDT))
        for _ in range(-reps):
            body()
    else:
        # 4 bodies per iteration: tile pools rotate across the group, so
        # the next rep's input DMAs land in an alternate frames buffer and
        # overlap compute; the For_i back-edge barrier amortizes 4x
        assert reps % 4 == 0
        nc.sync.dma_start(call[:], ch.ap().bitcast(MDT))
        with tc.For_i(0, reps // 4, 1):
            body()
            body()
            body()
            body()


def build_nc(mm_dt=MM_DT, reps: int = 1, variant: str = "full"):
    nc = bacc.Bacc("TRN2", target_bir_lowering=False, debug=False)
    xh = nc.dram_tensor("x", [BPC, T], FP32, kind="ExternalInput")
    ch = nc.dram_tensor("consts", [128, NCONST], FP32, kind="ExternalInput")
    outh = nc.dram_tensor("out", [BPC, 128, 5 * KOUT], FP32,
                          kind="ExternalOutput")
    with tile.TileContext(nc) as tc, ExitStack() as ctx:
        _build(ctx, tc, xh, ch, outh, mm_dt, reps, variant)
    nc.compile()
    return nc


def _round_f32r(a):
    """Round fp32 to f32r (1-8-11): round-to-nearest-even to 11 mantissa
    bits, low 12 bits zeroed. Matches HW DVE rounding (micro-verified)."""
    xi = np.ascontiguousarray(a, np.float32).view(np.uint32)
    lsb = np.uint32(1) << 12
    bias = (lsb >> 1) - 1 + ((xi >> 12) & 1)
    return ((xi + bias) & ~np.uint32(lsb - 1)).view(np.float32)


def make_in_maps(x, W_real, W_imag):
    xs = np.asarray(x, dtype=np.float32).reshape(B_FULL, T)
    Wr = np.asarray(W_real, np.float32)
    Wi = np.asarray(W_imag, np.float32)
    # unfolded bases: C[t, k] = Wr[k, t], S[t, k] = Wi[k, t], t = 128q+p
    wr_dev = np.ascontiguousarray(Wr[:KOUT, :].T)   # [512 t, 256 k]
    wi_dev = np.ascontiguousarray(Wi[:KOUT, :].T)
    if MM_DT == F32R:
        wr_dev = _round_f32r(wr_dev)
        wi_dev = _round_f32r(wi_dev)
    # pack [ident | cos(q,k) | sin(q,k)] as one [128, 2176] tensor where
    # chunk q of the 512-row basis maps to rows 128q+p
    consts = np.zeros((128, NCONST), np.float32)
    consts[:, 0:128] = np.eye(128, dtype=np.float32)
    consts[:, 128:1152] = wr_dev.reshape(4, 128, KOUT).transpose(
        1, 0, 2).reshape(128, 1024)
    consts[:, 1152:2176] = wi_dev.reshape(4, 128, KOUT).transpose(
        1, 0, 2).reshape(128, 1024)
    return [
        {"x": np.ascontiguousarray(xs[i * BPC:(i + 1) * BPC]),
         "consts": consts}
        for i in range(N_CORES)
    ]


_NC_CACHE = {}


def kernel(x, W_real, W_imag):
    key = (str(MM_DT), 1)
    if key not in _NC_CACHE:
        _NC_CACHE[key] = build_nc(MM_DT, 1)
    nc = _NC_CACHE[key]
    in_maps = make_in_maps(x, W_real, W_imag)
    res = run_bass_kernel_spmd(nc, in_maps, core_ids=list(range(N_CORES)))
    dev = np.concatenate([r["out"] for r in res.results], axis=0)
    dev = dev.reshape(B_FULL, 128, 5, KOUT)        # [b, p, blk, k]
    full = np.empty((B_FULL, F, KOUT), np.float32)
    blocks = dev.transpose(0, 2, 1, 3)             # [b, blk, p, k]
    full[:, 0:512] = blocks[:, 0:4].reshape(B_FULL, 512, KOUT)
    full[:, 512:601] = blocks[:, 4, 39:128]        # frames 473..600, keep 512+
    out = np.ascontiguousarray(full.transpose(0, 2, 1))  # [32, K, F]
    return out.reshape(B_FULL, C_FULL, KOUT, F).astype(np.float32)
